# revision 15
# baseline (speedup 1.0000x reference)
"""Multi-head causal attention on 8 Trainium2 NeuronCores.

Sharding: core = (batch b in {0,1}) x (head-group g in {0..3}); each core
computes 4 of the 16 heads for one batch element and returns a partial
(d_model, n) output (its heads' contribution to the final projection).
Host sums the 4 partials per batch (w_o row-parallel reduce), transposes,
and stacks.

Single woven stream (v3): the attention j-stream is ScalarE-paced (one
exp per step covers both heads of a pair, ~1.1us) while its own matmuls
need only ~0.6us of PE — so ALL projection work that the baseline ran as
a separate PE-only phase is now fed into that slack through two
deadline-driven filler queues:

  DMA order: xk, xv, xq (so the last-arriving input gates only qp[0]).
  startup:   kh[0] i0..3 + qp[0] i0 (k-outer per i-chunk, 2 rotating
             psX banks), then the (p=0, c, J) stream begins.
  pre_q:     qp[0] i1..3, kh[1]/qp[1] i-chunks — pumped BEFORE the
             scores of their deadline step.
  post_q:    V path — vh[m] transposed projection i-chunks (arrival-
             driven on xv, identical shape to kh) and PE-transpose units
             converting vh 128x128 blocks into the natural augmented
             va[nt] = [128 j, (V_h | 1) x 4] layout (f32 psum transpose,
             DVE evac).  Pumped AFTER each step's exp, giving the AV
             flush _AVDEPTH steps of slack.
  phase 3:   output projection per i-chunk interleaved into the p=1
             stream (unchanged), sharing the psX pool.
  PSUM:      scores 2x2 banks + AV accumulators 2 + shared psX 2 = 8.

This keeps the PE continuously busy (no HAM re-throttle) and makes the
kernel PE-bound instead of phase-serialized (baseline alternated a
PE-only projection phase with an exp-gated attention phase).

Attention math per step (p, c, J) is unchanged from the baseline:
scores for BOTH heads of a pair via concurrent row-tiled K=64 matmuls
into one [128,1024] psum; causal mask accumulated on the diagonal strip
by a tri x mskb matmul; ONE exp activation per step covering both heads;
AV via [V_h | 1]-augmented matmuls (denominator for free) software-
pipelined _AVDEPTH steps behind scores; normalize (reciprocal+broadcast+
mul) per pass end on DVE/Pool.
"""

import math
import os

import numpy as np

H = 16
D_MODEL = 1024
D_K = 64
N = 2048
B = 2
N_CORES = 8
N_GROUPS = 4          # head groups (tensor parallel)
HPC = H // N_GROUPS   # heads per core = 4
GD = HPC * D_K        # group output dim = 256
EXP_SCALE = 1.0 / math.sqrt(D_K)
EXP_BIAS = -5.0

_DT = os.environ.get("BASS_MHA_DT", "bf16")
_OUT_DT = os.environ.get("BASS_MHA_OUTDT", "bf16")
_AVDEPTH = int(os.environ.get("BASS_MHA_AVD", "2"))
_PACE = int(os.environ.get("BASS_MHA_PACE", "0"))

KC = D_MODEL // 128   # 8 contraction chunks
NI = N // 512         # 4 i-chunks of 512
NJ = N // 128         # 16 j-chunks of 128


def _build(dt_name: str, n_iters: int = 1):
    """Emit and compile the single-core SPMD program. Returns compiled nc."""
    import concourse.bacc as bacc
    import concourse.mybir as mybir
    import concourse.tile as tile
    from concourse.ap import AP

    dt = {"bf16": mybir.dt.bfloat16, "f32r": mybir.dt.float32r}[dt_name]
    f32 = mybir.dt.float32
    odt = {"bf16": mybir.dt.bfloat16, "f32": f32}[_OUT_DT]

    nc = bacc.Bacc("TRN2", num_devices=N_CORES)

    xqT = nc.dram_tensor("xqT", [D_MODEL, N], dt, kind="ExternalInput").ap()
    xkT = nc.dram_tensor("xkT", [D_MODEL, N], dt, kind="ExternalInput").ap()
    xvT = nc.dram_tensor("xvT", [D_MODEL, N], dt, kind="ExternalInput").ap()
    wqT = nc.dram_tensor("wqT", [D_MODEL, GD], dt, kind="ExternalInput").ap()
    wkT = nc.dram_tensor("wkT", [D_MODEL, GD], dt, kind="ExternalInput").ap()
    wvT = nc.dram_tensor("wvT", [D_MODEL, GD], dt, kind="ExternalInput").ap()
    woT = nc.dram_tensor("woT", [GD, D_MODEL], dt, kind="ExternalInput").ap()
    tri = nc.dram_tensor("tri", [128, 128], dt, kind="ExternalInput").ap()
    mskb = nc.dram_tensor("mskb", [128, 128], dt, kind="ExternalInput").ap()
    outT = nc.dram_tensor("outT", [D_MODEL, N], odt, kind="ExternalOutput").ap()

    xq_b = xqT.rearrange("(kc p) i -> p kc i", p=128)
    xk_b = xkT.rearrange("(kc p) i -> p kc i", p=128)
    xv_b = xvT.rearrange("(kc p) i -> p kc i", p=128)
    wq_b = wqT.rearrange("(kc p) m -> p kc m", p=128)
    wk_b = wkT.rearrange("(kc p) m -> p kc m", p=128)
    wv_b = wvT.rearrange("(kc p) m -> p kc m", p=128)
    wo_b = woT.rearrange("(oc p) m -> p oc m", p=128)

    from contextlib import ExitStack

    with tile.TileContext(nc) as tc, ExitStack() as ctx:
        sb_w = ctx.enter_context(tc.tile_pool(name="weights", bufs=1))
        sb_x = ctx.enter_context(tc.tile_pool(name="xin", bufs=1))
        sb_p = ctx.enter_context(tc.tile_pool(name="persist", bufs=1))
        sb_e = ctx.enter_context(tc.tile_pool(name="expw", bufs=4))
        sb_o = ctx.enter_context(tc.tile_pool(name="outw", bufs=2))

        def body():
            # ---- resident weights (scalar=qActDynamicHW ring) ----
            wqb = sb_w.tile([128, KC * GD], dt, tag="wqb", name="wqb")
            wkb = sb_w.tile([128, KC * GD], dt, tag="wkb", name="wkb")
            wvb = sb_w.tile([128, KC * GD], dt, tag="wvb", name="wvb")
            wob = sb_w.tile([128, 2 * D_MODEL], dt, tag="wob", name="wob")
            tri_s = sb_w.tile([128, 128], dt, tag="tri", name="tri")
            mskb_s = sb_w.tile([128, 128], dt, tag="mskb", name="mskb")
            ebias = sb_w.tile([128, 1], f32, tag="ebias", name="ebias")
            nc.vector.memset(ebias[:], EXP_BIAS)
            # one batched DMA per tensor: [p, (kc m)] gather of all chunks
            nc.scalar.dma_start(
                wkb[:].rearrange("p (kc m) -> p kc m", kc=KC), wk_b)
            nc.scalar.dma_start(
                wvb[:].rearrange("p (kc m) -> p kc m", kc=KC), wv_b)
            nc.scalar.dma_start(
                wqb[:].rearrange("p (kc m) -> p kc m", kc=KC), wq_b)
            nc.scalar.dma_start(
                wob[:].rearrange("p (oc m) -> p oc m", oc=2), wo_b)
            nc.scalar.dma_start(tri_s[:], tri[:])
            nc.scalar.dma_start(mskb_s[:], mskb[:])
            wk_s = [wkb[:, k * GD:(k + 1) * GD] for k in range(KC)]
            wq_s = [wqb[:, k * GD:(k + 1) * GD] for k in range(KC)]
            wv_s = [wvb[:, k * GD:(k + 1) * GD] for k in range(KC)]
            wo_s = [wob[:, o * D_MODEL:(o + 1) * D_MODEL] for o in range(2)]

            # ---- per-chunk x inputs (sync ring, in consumption order) ----
            xkb = sb_x.tile([128, KC * N], dt, tag="xkb", name="xkb")
            xqb = sb_x.tile([128, KC * N], dt, tag="xqb", name="xqb")
            xvb = sb_x.tile([128, KC * N], dt, tag="xvb", name="xvb")
            nc.sync.dma_start(
                xkb[:].rearrange("p (kc i) -> p kc i", kc=KC), xk_b)
            nc.sync.dma_start(
                xvb[:].rearrange("p (kc i) -> p kc i", kc=KC), xv_b)
            nc.sync.dma_start(
                xqb[:].rearrange("p (kc i) -> p kc i", kc=KC), xq_b)
            xk_c = [xkb[:, k * N:(k + 1) * N] for k in range(KC)]
            xq_c = [xqb[:, k * N:(k + 1) * N] for k in range(KC)]
            xv_c = [xvb[:, k * N:(k + 1) * N] for k in range(KC)]

            # ---- persistent intermediates ----
            # transposed pair tiles: [(h_even d64 | h_odd d64), n]
            kh = [sb_p.tile([128, N], dt, tag=f"kh{m}", name=f"kh{m}") for m in range(2)]
            qp = [sb_p.tile([128, N], dt, tag=f"qp{m}", name=f"qp{m}") for m in range(2)]
            # V natural augmented per j-chunk: [128 j, (V_h | 1) x 4]
            va = [sb_p.tile([128, 4 * 65], dt, tag=f"va{nt}", name=f"va{nt}") for nt in range(NJ)]
            # normalized O^T per pair: [(h_even d64 | h_odd d64), n]
            ot = [sb_p.tile([128, N], dt, tag=f"ot{p}", name=f"ot{p}") for p in range(2)]

            for nt in range(NJ):
                for h in range(HPC):
                    nc.vector.memset(va[nt][:, h * 65 + 64:h * 65 + 65], 1.0)

            with tc.tile_pool(name="psS", bufs=2, space="PSUM") as psS, \
                 tc.tile_pool(name="psA", bufs=1, space="PSUM") as psA, \
                 tc.tile_pool(name="psX", bufs=2, space="PSUM") as psX:

                # ---------- filler units (all use the shared psX pool) ----
                def proj_chunk(w_s, x_c, dst, m, i):
                    # one i-chunk of a transposed projection m-pass
                    px = psX.tile([128, 512], f32, tag="px", name="px")
                    for k in range(KC):
                        nc.tensor.matmul(
                            px[:],
                            w_s[k][:, m * 128:(m + 1) * 128],
                            x_c[k][:, i * 512:(i + 1) * 512],
                            start=(k == 0), stop=(k == KC - 1),
                        )
                    nc.vector.tensor_copy(
                        dst[m][:, i * 512:(i + 1) * 512], px[:])

                def v_chunk(nt):
                    # natural-layout col-tiled m64 V projection for j-chunk nt
                    px = psX.tile([128, 512], f32, tag="px", name="px")
                    for k in range(KC):
                        for u in range(2):
                            nc.tensor.matmul(
                                px[64 * u:64 * (u + 1), 0:256],
                                xv_c[k][:, nt * 128 + 64 * u:
                                        nt * 128 + 64 * (u + 1)],
                                wv_s[k][:],
                                start=(k == 0), stop=(k == KC - 1),
                                tile_position=(0, 64 * u),
                                skip_group_check=True,
                            )
                    for h in range(HPC):
                        nc.vector.tensor_copy(
                            va[nt][:, h * 65:h * 65 + 64],
                            px[:, h * 64:(h + 1) * 64])

                def emit_phase3(c):
                    # output projection for i-chunk c (both pairs' ot ready)
                    us = sb_o.tile([128, 8 * 512], odt, tag="us", name="us")
                    for ms in range(8):
                        pu = psX.tile([128, 512], f32, tag="px", name="px")
                        for p in range(2):
                            nc.tensor.matmul(
                                pu[:],
                                wo_s[p][:, ms * 128:(ms + 1) * 128],
                                ot[p][:, c * 512:(c + 1) * 512],
                                start=(p == 0), stop=(p == 1),
                            )
                        nc.vector.tensor_copy(
                            us[:, ms * 512:(ms + 1) * 512], pu[:])
                    nc.scalar.dma_start(
                        AP(outT.tensor, c * 512,
                           [[N, 128], [128 * N, 8], [1, 512]]),
                        us[:],
                    )

                # ---------- filler queues ----------
                # (deadline (p,c,J), emit_fn); deadlines are hand-spread so
                # roughly one unit lands per step, always at or before the
                # unit's hard consumption deadline.
                def _pj(w_s, x_c, dst, m, i):
                    return lambda: proj_chunk(w_s, x_c, dst, m, i)

                # pre_q units must land BEFORE the scores of their deadline
                # step (they produce kh/qp operands of that step).  kh[1]
                # units depend only on xk (first to arrive) so they get the
                # earliest deadlines — in a cold start they fill the PE
                # while the stream waits on xq; in the loop steady state
                # the stream paces them into its slack.
                pre_q = [
                    ((0, 0, 0), _pj(wk_s, xk_c, kh, 1, 0)),
                    ((0, 0, 2), _pj(wk_s, xk_c, kh, 1, 1)),
                    ((0, 1, 0), _pj(wq_s, xq_c, qp, 0, 1)),
                    ((0, 1, 2), _pj(wk_s, xk_c, kh, 1, 2)),
                    ((0, 1, 6), _pj(wk_s, xk_c, kh, 1, 3)),
                    ((0, 2, 0), _pj(wq_s, xq_c, qp, 0, 2)),
                    ((0, 2, 4), _pj(wq_s, xq_c, qp, 1, 0)),
                    ((0, 3, 0), _pj(wq_s, xq_c, qp, 0, 3)),
                    ((0, 3, 6), _pj(wq_s, xq_c, qp, 1, 1)),
                    ((1, 1, 0), _pj(wq_s, xq_c, qp, 1, 2)),
                    ((1, 2, 0), _pj(wq_s, xq_c, qp, 1, 3)),
                ]
                # post_q units are needed only by the AV flush _AVDEPTH
                # steps later, so they are pumped after the step's exp.
                # va[nt] is first consumed by the flush of step
                # (0, c_min(nt), nt); deadlines sit at or before that.
                post_q = [
                    ((0, 0, 0), lambda: v_chunk(2)),
                    ((0, 0, 1), lambda: v_chunk(3)),
                    ((0, 0, 2), lambda: v_chunk(4)),
                    ((0, 0, 3), lambda: v_chunk(5)),
                    ((0, 1, 1), lambda: v_chunk(6)),
                    ((0, 1, 3), lambda: v_chunk(7)),
                    ((0, 1, 5), lambda: v_chunk(8)),
                    ((0, 1, 7), lambda: v_chunk(9)),
                    ((0, 2, 2), lambda: v_chunk(10)),
                    ((0, 2, 5), lambda: v_chunk(11)),
                    ((0, 2, 8), lambda: v_chunk(12)),
                    ((0, 2, 11), lambda: v_chunk(13)),
                    ((0, 3, 4), lambda: v_chunk(14)),
                    ((0, 3, 9), lambda: v_chunk(15)),
                ]

                def pump(q, key, extra=0):
                    # emit all units due at `key`, plus up to `extra` more
                    n = 0
                    while q and (q[0][0] <= key or n < extra):
                        if q[0][0] > key:
                            n += 1
                        q.pop(0)[1]()

                # ---------- startup ----------
                for i in range(NI):
                    proj_chunk(wk_s, xk_c, kh, 0, i)
                v_chunk(0)
                v_chunk(1)
                proj_chunk(wq_s, xq_c, qp, 0, 0)

                # ---------- attention stream ----------
                pending = []    # [(p, c, J, et, w, po_e, po_o)]
                ph3_q = []      # i-chunks whose output projection is due

                def flush():
                    if not pending:
                        return
                    p, c, J, et, w, po_e, po_o = pending.pop(0)
                    off = 512 - w
                    for e, po in ((0, po_e), (1, po_o)):
                        h = 2 * p + e
                        lo = (512 - w) if e == 0 else 512
                        nc.tensor.matmul(
                            po[:, off:512],
                            va[J][:, h * 65:h * 65 + 65],
                            et[:, lo:lo + w],
                            start=(J == 0), stop=(J == 4 * c + 3),
                            skip_group_check=True,
                        )
                    if J == 4 * c + 3:
                        # normalize chunk c for both heads of pair p
                        for e, po in ((0, po_e), (1, po_o)):
                            rec = sb_o.tile([1, 512], f32, tag="rec", name="rec")
                            nc.vector.reciprocal(rec[:], po[64:65, :])
                            rb = sb_o.tile([64, 512], f32, tag="rb", name="rb")
                            nc.gpsimd.partition_broadcast(rb[:], rec[0:1, :])
                            nc.vector.tensor_mul(
                                ot[p][64 * e:64 * (e + 1),
                                      c * 512:(c + 1) * 512],
                                po[0:64, :], rb[:],
                            )
                        if p == 1:
                            ph3_q.append(c)
                    elif ph3_q:
                        emit_phase3(ph3_q.pop(0))

                for p in range(2):
                    for c in range(NI):
                        po_e = psA.tile([65, 512], f32, tag="poe", name="poe")
                        po_o = psA.tile([65, 512], f32, tag="poo", name="poo")
                        for J in range(4 * c + 4):
                            pump(pre_q, (p, c, J), _PACE)
                            diag = (J // 4 == c)
                            s = J % 4
                            w = 512 - 128 * s if diag else 512
                            off = 128 * s if diag else 0
                            sc = psS.tile([128, 1024], f32, tag="sc", name="sc")
                            for e in range(2):
                                nc.tensor.matmul(
                                    sc[:, (512 - w) if e == 0 else 512:
                                       512 if e == 0 else 512 + w],
                                    kh[p][64 * e:64 * (e + 1),
                                          J * 128:(J + 1) * 128],
                                    qp[p][64 * e:64 * (e + 1),
                                          c * 512 + off:(c + 1) * 512],
                                    start=True, stop=not diag,
                                    skip_group_check=True,
                                )
                            if diag:
                                # causal: accumulate -240*(j-i) on diag strip
                                nc.tensor.matmul(
                                    sc[:, 512 - w:512 - w + 128],
                                    tri_s[:], mskb_s[:],
                                    start=False, stop=True,
                                    skip_group_check=True,
                                )
                                nc.tensor.matmul(
                                    sc[:, 512:512 + 128],
                                    tri_s[:], mskb_s[:],
                                    start=False, stop=True,
                                    skip_group_check=True,
                                )
                            et = sb_e.tile([128, 1024], dt, tag="et", name="et")
                            nc.scalar.activation(
                                et[:, 512 - w:512 - w + 2 * w],
                                sc[:, 512 - w:512 - w + 2 * w],
                                mybir.ActivationFunctionType.Exp,
                                bias=ebias[:], scale=EXP_SCALE,
                            )
                            pump(post_q, (p, c, J), _PACE)
                            if len(pending) >= _AVDEPTH:
                                flush()
                            pending.append((p, c, J, et, w, po_e, po_o))
                while pre_q:
                    pre_q.pop(0)[1]()
                while post_q:
                    post_q.pop(0)[1]()
                while pending:
                    flush()
                while ph3_q:
                    emit_phase3(ph3_q.pop(0))

        if n_iters > 1 and os.environ.get("BASS_MHA_UNROLL", "0") == "1":
            for _ in range(n_iters):
                body()
        elif n_iters > 1:
            with tc.For_i(0, n_iters, 1):
                body()
        else:
            body()

    nc.compile()
    return nc


_CACHE = {}


def _get_program(dt_name: str, n_iters: int = 1):
    key = (dt_name, n_iters, _OUT_DT, _AVDEPTH, _PACE)
    if key not in _CACHE:
        _CACHE[key] = _build(dt_name, n_iters)
    return _CACHE[key]


def _np_dt(dt_name: str):
    if dt_name == "bf16":
        import ml_dtypes
        return ml_dtypes.bfloat16
    return np.float32


def make_in_maps(q, k, v, w_q, w_k, w_v, w_o, dt_name: str):
    """Build the 8 per-core input dicts (host-side shard + transpose)."""
    ndt = _np_dt(dt_name)
    tri = np.triu(np.ones((128, 128), np.float32)).astype(ndt)
    mskb = (-240.0 * np.tril(np.ones((128, 128), np.float32), -1)).astype(ndt)
    in_maps = []
    for b in range(B):
        xqT = np.ascontiguousarray(q[b].T).astype(ndt)
        xkT = np.ascontiguousarray(k[b].T).astype(ndt)
        xvT = np.ascontiguousarray(v[b].T).astype(ndt)
        for g in range(N_GROUPS):
            r0 = GD * g
            in_maps.append({
                "xqT": xqT,
                "xkT": xkT,
                "xvT": xvT,
                "wqT": np.ascontiguousarray(w_q[r0:r0 + GD, :].T).astype(ndt),
                "wkT": np.ascontiguousarray(w_k[r0:r0 + GD, :].T).astype(ndt),
                "wvT": np.ascontiguousarray(w_v[r0:r0 + GD, :].T).astype(ndt),
                "woT": np.ascontiguousarray(w_o[:, r0:r0 + GD].T).astype(ndt),
                "tri": tri,
                "mskb": mskb,
            })
    return in_maps


def kernel(q, k, v, w_q, w_k, w_v, w_o):
    from concourse.bass_utils import run_bass_kernel_spmd

    dt_name = _DT
    nc = _get_program(dt_name)
    in_maps = make_in_maps(q, k, v, w_q, w_k, w_v, w_o, dt_name)
    res = run_bass_kernel_spmd(nc, in_maps, core_ids=list(range(N_CORES)))
    parts = [np.asarray(res.results[i]["outT"], dtype=np.float32)
             for i in range(N_CORES)]
    out = np.empty((B, N, D_MODEL), np.float32)
    for b in range(B):
        acc = parts[N_GROUPS * b]
        for g in range(1, N_GROUPS):
            acc = acc + parts[N_GROUPS * b + g]
        out[b] = acc.T
    return out


# revision 16
# speedup vs baseline: 1.0420x; 1.0420x over previous
"""Multi-head causal attention on 8 Trainium2 NeuronCores.

Sharding: core = (batch b in {0,1}) x (head-group g in {0..3}); each core
computes 4 of the 16 heads for one batch element and returns a partial
(d_model, n) output (its heads' contribution to the final projection).
Host sums the 4 partials per batch (w_o row-parallel reduce), transposes,
and stacks.

Single woven stream: the attention j-stream is ScalarE-paced (one exp
per step covers both heads of a pair, ~1.1us) while its own matmuls
need only ~0.6us of PE — so ALL projection work that the baseline ran
as a separate PE-only phase is fed into that slack through two
deadline-driven filler queues:

  DMA:       ONE batched 3D-AP DMA per tensor (x inputs on the sync
             ring in order xk, xv, xq so the last-arriving input gates
             only qp[0]; weights on the scalar ring) — 9 dma_starts per
             iteration instead of 55, amortizing the ~2us per-DMA
             completion latency.
  startup:   kh[0] i0..3, va[0..1], qp[0] i0 (k-outer per i-chunk, 2
             rotating psX banks), then the (p=0, c, J) stream begins.
  pre_q:     qp[0] i1..3, kh[1]/qp[1] i-chunks — pumped BEFORE the
             scores of their deadline step (kh[1] gets the earliest
             deadlines: it depends only on xk, so in a cold start it
             fills the PE while the stream waits on xq).
  post_q:    remaining va[nt] m64 V-projection chunks — needed only by
             the AV flush _AVDEPTH steps later, so pumped AFTER each
             step's exp.
  phase 3:   output projection per i-chunk interleaved into the p=1
             stream, sharing the psX pool.
  PSUM:      scores 2x2 banks + AV accumulators 2 + shared psX 2 = 8.

This keeps the PE continuously busy (no HAM re-throttle) instead of
alternating a PE-only projection phase with an exp-gated attention
phase.  Measured (loop-amortized, device-resident inputs): baseline
247.3us/iter -> 242us/iter; the stream itself is PE-dense in the
timeline sim (97% busy between startup and tail).

Attention math per step (p, c, J) is unchanged from the baseline:
scores for BOTH heads of a pair via concurrent row-tiled K=64 matmuls
into one [128,1024] psum; causal mask accumulated on the diagonal strip
by a tri x mskb matmul; ONE exp activation per step covering both heads;
AV via [V_h | 1]-augmented matmuls (denominator for free) software-
pipelined _AVDEPTH steps behind scores; normalize (reciprocal+broadcast+
mul) per pass end on DVE/Pool.
"""

import math
import os

import numpy as np

H = 16
D_MODEL = 1024
D_K = 64
N = 2048
B = 2
N_CORES = 8
N_GROUPS = 4          # head groups (tensor parallel)
HPC = H // N_GROUPS   # heads per core = 4
GD = HPC * D_K        # group output dim = 256
EXP_SCALE = 1.0 / math.sqrt(D_K)
EXP_BIAS = -5.0

_DT = os.environ.get("BASS_MHA_DT", "bf16")
_OUT_DT = os.environ.get("BASS_MHA_OUTDT", "bf16")
_AVDEPTH = int(os.environ.get("BASS_MHA_AVD", "2"))
_PACE = int(os.environ.get("BASS_MHA_PACE", "0"))

KC = D_MODEL // 128   # 8 contraction chunks
NI = N // 512         # 4 i-chunks of 512
NJ = N // 128         # 16 j-chunks of 128


def _build(dt_name: str, n_iters: int = 1):
    """Emit and compile the single-core SPMD program. Returns compiled nc."""
    import concourse.bacc as bacc
    import concourse.mybir as mybir
    import concourse.tile as tile
    from concourse.ap import AP

    dt = {"bf16": mybir.dt.bfloat16, "f32r": mybir.dt.float32r}[dt_name]
    f32 = mybir.dt.float32
    odt = {"bf16": mybir.dt.bfloat16, "f32": f32}[_OUT_DT]

    nc = bacc.Bacc("TRN2", num_devices=N_CORES)

    xqT = nc.dram_tensor("xqT", [D_MODEL, N], dt, kind="ExternalInput").ap()
    xkT = nc.dram_tensor("xkT", [D_MODEL, N], dt, kind="ExternalInput").ap()
    xvT = nc.dram_tensor("xvT", [D_MODEL, N], dt, kind="ExternalInput").ap()
    wqT = nc.dram_tensor("wqT", [D_MODEL, GD], dt, kind="ExternalInput").ap()
    wkT = nc.dram_tensor("wkT", [D_MODEL, GD], dt, kind="ExternalInput").ap()
    wvT = nc.dram_tensor("wvT", [D_MODEL, GD], dt, kind="ExternalInput").ap()
    woT = nc.dram_tensor("woT", [GD, D_MODEL], dt, kind="ExternalInput").ap()
    tri = nc.dram_tensor("tri", [128, 128], dt, kind="ExternalInput").ap()
    mskb = nc.dram_tensor("mskb", [128, 128], dt, kind="ExternalInput").ap()
    outT = nc.dram_tensor("outT", [D_MODEL, N], odt, kind="ExternalOutput").ap()

    xq_b = xqT.rearrange("(kc p) i -> p kc i", p=128)
    xk_b = xkT.rearrange("(kc p) i -> p kc i", p=128)
    xv_b = xvT.rearrange("(kc p) i -> p kc i", p=128)
    wq_b = wqT.rearrange("(kc p) m -> p kc m", p=128)
    wk_b = wkT.rearrange("(kc p) m -> p kc m", p=128)
    wv_b = wvT.rearrange("(kc p) m -> p kc m", p=128)
    wo_b = woT.rearrange("(oc p) m -> p oc m", p=128)

    from contextlib import ExitStack

    with tile.TileContext(nc) as tc, ExitStack() as ctx:
        sb_w = ctx.enter_context(tc.tile_pool(name="weights", bufs=1))
        sb_x = ctx.enter_context(tc.tile_pool(name="xin", bufs=1))
        sb_p = ctx.enter_context(tc.tile_pool(name="persist", bufs=1))
        sb_e = ctx.enter_context(tc.tile_pool(name="expw", bufs=4))
        sb_o = ctx.enter_context(tc.tile_pool(name="outw", bufs=2))

        def body():
            # ---- resident weights (scalar=qActDynamicHW ring) ----
            wqb = sb_w.tile([128, KC * GD], dt, tag="wqb", name="wqb")
            wkb = sb_w.tile([128, KC * GD], dt, tag="wkb", name="wkb")
            wvb = sb_w.tile([128, KC * GD], dt, tag="wvb", name="wvb")
            wob = sb_w.tile([128, 2 * D_MODEL], dt, tag="wob", name="wob")
            tri_s = sb_w.tile([128, 128], dt, tag="tri", name="tri")
            mskb_s = sb_w.tile([128, 128], dt, tag="mskb", name="mskb")
            ebias = sb_w.tile([128, 1], f32, tag="ebias", name="ebias")
            nc.vector.memset(ebias[:], EXP_BIAS)
            # one batched DMA per tensor: [p, (kc m)] gather of all chunks
            nc.scalar.dma_start(
                wkb[:].rearrange("p (kc m) -> p kc m", kc=KC), wk_b)
            nc.scalar.dma_start(
                wvb[:].rearrange("p (kc m) -> p kc m", kc=KC), wv_b)
            nc.scalar.dma_start(
                wqb[:].rearrange("p (kc m) -> p kc m", kc=KC), wq_b)
            nc.scalar.dma_start(
                wob[:].rearrange("p (oc m) -> p oc m", oc=2), wo_b)
            nc.scalar.dma_start(tri_s[:], tri[:])
            nc.scalar.dma_start(mskb_s[:], mskb[:])
            wk_s = [wkb[:, k * GD:(k + 1) * GD] for k in range(KC)]
            wq_s = [wqb[:, k * GD:(k + 1) * GD] for k in range(KC)]
            wv_s = [wvb[:, k * GD:(k + 1) * GD] for k in range(KC)]
            wo_s = [wob[:, o * D_MODEL:(o + 1) * D_MODEL] for o in range(2)]

            # ---- per-chunk x inputs (sync ring, in consumption order) ----
            xkb = sb_x.tile([128, KC * N], dt, tag="xkb", name="xkb")
            xqb = sb_x.tile([128, KC * N], dt, tag="xqb", name="xqb")
            xvb = sb_x.tile([128, KC * N], dt, tag="xvb", name="xvb")
            nc.sync.dma_start(
                xkb[:].rearrange("p (kc i) -> p kc i", kc=KC), xk_b)
            nc.sync.dma_start(
                xvb[:].rearrange("p (kc i) -> p kc i", kc=KC), xv_b)
            nc.sync.dma_start(
                xqb[:].rearrange("p (kc i) -> p kc i", kc=KC), xq_b)
            xk_c = [xkb[:, k * N:(k + 1) * N] for k in range(KC)]
            xq_c = [xqb[:, k * N:(k + 1) * N] for k in range(KC)]
            xv_c = [xvb[:, k * N:(k + 1) * N] for k in range(KC)]

            # ---- persistent intermediates ----
            # transposed pair tiles: [(h_even d64 | h_odd d64), n]
            kh = [sb_p.tile([128, N], dt, tag=f"kh{m}", name=f"kh{m}") for m in range(2)]
            qp = [sb_p.tile([128, N], dt, tag=f"qp{m}", name=f"qp{m}") for m in range(2)]
            # V natural augmented per j-chunk: [128 j, (V_h | 1) x 4]
            va = [sb_p.tile([128, 4 * 65], dt, tag=f"va{nt}", name=f"va{nt}") for nt in range(NJ)]
            # normalized O^T per pair: [(h_even d64 | h_odd d64), n]
            ot = [sb_p.tile([128, N], dt, tag=f"ot{p}", name=f"ot{p}") for p in range(2)]

            for nt in range(NJ):
                for h in range(HPC):
                    nc.vector.memset(va[nt][:, h * 65 + 64:h * 65 + 65], 1.0)

            with tc.tile_pool(name="psS", bufs=2, space="PSUM") as psS, \
                 tc.tile_pool(name="psA", bufs=1, space="PSUM") as psA, \
                 tc.tile_pool(name="psX", bufs=2, space="PSUM") as psX:

                # ---------- filler units (all use the shared psX pool) ----
                def proj_chunk(w_s, x_c, dst, m, i):
                    # one i-chunk of a transposed projection m-pass
                    px = psX.tile([128, 512], f32, tag="px", name="px")
                    for k in range(KC):
                        nc.tensor.matmul(
                            px[:],
                            w_s[k][:, m * 128:(m + 1) * 128],
                            x_c[k][:, i * 512:(i + 1) * 512],
                            start=(k == 0), stop=(k == KC - 1),
                        )
                    nc.vector.tensor_copy(
                        dst[m][:, i * 512:(i + 1) * 512], px[:])

                def v_chunk(nt):
                    # natural-layout col-tiled m64 V projection for j-chunk nt
                    px = psX.tile([128, 512], f32, tag="px", name="px")
                    for k in range(KC):
                        for u in range(2):
                            nc.tensor.matmul(
                                px[64 * u:64 * (u + 1), 0:256],
                                xv_c[k][:, nt * 128 + 64 * u:
                                        nt * 128 + 64 * (u + 1)],
                                wv_s[k][:],
                                start=(k == 0), stop=(k == KC - 1),
                                tile_position=(0, 64 * u),
                                skip_group_check=True,
                            )
                    for h in range(HPC):
                        nc.vector.tensor_copy(
                            va[nt][:, h * 65:h * 65 + 64],
                            px[:, h * 64:(h + 1) * 64])

                def emit_phase3(c):
                    # output projection for i-chunk c (both pairs' ot ready)
                    us = sb_o.tile([128, 8 * 512], odt, tag="us", name="us")
                    for ms in range(8):
                        pu = psX.tile([128, 512], f32, tag="px", name="px")
                        for p in range(2):
                            nc.tensor.matmul(
                                pu[:],
                                wo_s[p][:, ms * 128:(ms + 1) * 128],
                                ot[p][:, c * 512:(c + 1) * 512],
                                start=(p == 0), stop=(p == 1),
                            )
                        nc.vector.tensor_copy(
                            us[:, ms * 512:(ms + 1) * 512], pu[:])
                    nc.scalar.dma_start(
                        AP(outT.tensor, c * 512,
                           [[N, 128], [128 * N, 8], [1, 512]]),
                        us[:],
                    )

                # ---------- filler queues ----------
                # (deadline (p,c,J), emit_fn); deadlines are hand-spread so
                # roughly one unit lands per step, always at or before the
                # unit's hard consumption deadline.
                def _pj(w_s, x_c, dst, m, i):
                    return lambda: proj_chunk(w_s, x_c, dst, m, i)

                # pre_q units must land BEFORE the scores of their deadline
                # step (they produce kh/qp operands of that step).  kh[1]
                # units depend only on xk (first to arrive) so they get the
                # earliest deadlines — in a cold start they fill the PE
                # while the stream waits on xq; in the loop steady state
                # the stream paces them into its slack.
                pre_q = [
                    ((0, 0, 0), _pj(wk_s, xk_c, kh, 1, 0)),
                    ((0, 0, 2), _pj(wk_s, xk_c, kh, 1, 1)),
                    ((0, 1, 0), _pj(wq_s, xq_c, qp, 0, 1)),
                    ((0, 1, 2), _pj(wk_s, xk_c, kh, 1, 2)),
                    ((0, 1, 6), _pj(wk_s, xk_c, kh, 1, 3)),
                    ((0, 2, 0), _pj(wq_s, xq_c, qp, 0, 2)),
                    ((0, 2, 4), _pj(wq_s, xq_c, qp, 1, 0)),
                    ((0, 3, 0), _pj(wq_s, xq_c, qp, 0, 3)),
                    ((0, 3, 6), _pj(wq_s, xq_c, qp, 1, 1)),
                    ((1, 1, 0), _pj(wq_s, xq_c, qp, 1, 2)),
                    ((1, 2, 0), _pj(wq_s, xq_c, qp, 1, 3)),
                ]
                # post_q units are needed only by the AV flush _AVDEPTH
                # steps later, so they are pumped after the step's exp.
                # va[nt] is first consumed by the flush of step
                # (0, c_min(nt), nt); deadlines sit at or before that.
                post_q = [
                    ((0, 0, 0), lambda: v_chunk(2)),
                    ((0, 0, 1), lambda: v_chunk(3)),
                    ((0, 0, 2), lambda: v_chunk(4)),
                    ((0, 0, 3), lambda: v_chunk(5)),
                    ((0, 1, 1), lambda: v_chunk(6)),
                    ((0, 1, 3), lambda: v_chunk(7)),
                    ((0, 1, 5), lambda: v_chunk(8)),
                    ((0, 1, 7), lambda: v_chunk(9)),
                    ((0, 2, 2), lambda: v_chunk(10)),
                    ((0, 2, 5), lambda: v_chunk(11)),
                    ((0, 2, 8), lambda: v_chunk(12)),
                    ((0, 2, 11), lambda: v_chunk(13)),
                    ((0, 3, 4), lambda: v_chunk(14)),
                    ((0, 3, 9), lambda: v_chunk(15)),
                ]

                def pump(q, key, extra=0):
                    # emit all units due at `key`, plus up to `extra` more
                    n = 0
                    while q and (q[0][0] <= key or n < extra):
                        if q[0][0] > key:
                            n += 1
                        q.pop(0)[1]()

                # ---------- startup ----------
                for i in range(NI):
                    proj_chunk(wk_s, xk_c, kh, 0, i)
                v_chunk(0)
                v_chunk(1)
                proj_chunk(wq_s, xq_c, qp, 0, 0)

                # ---------- attention stream ----------
                pending = []    # [(p, c, J, et, w, po_e, po_o)]
                ph3_q = []      # i-chunks whose output projection is due

                def flush():
                    if not pending:
                        return
                    p, c, J, et, w, po_e, po_o = pending.pop(0)
                    off = 512 - w
                    for e, po in ((0, po_e), (1, po_o)):
                        h = 2 * p + e
                        lo = (512 - w) if e == 0 else 512
                        nc.tensor.matmul(
                            po[:, off:512],
                            va[J][:, h * 65:h * 65 + 65],
                            et[:, lo:lo + w],
                            start=(J == 0), stop=(J == 4 * c + 3),
                            skip_group_check=True,
                        )
                    if J == 4 * c + 3:
                        # normalize chunk c for both heads of pair p
                        for e, po in ((0, po_e), (1, po_o)):
                            rec = sb_o.tile([1, 512], f32, tag="rec", name="rec")
                            nc.vector.reciprocal(rec[:], po[64:65, :])
                            rb = sb_o.tile([64, 512], f32, tag="rb", name="rb")
                            nc.gpsimd.partition_broadcast(rb[:], rec[0:1, :])
                            nc.vector.tensor_mul(
                                ot[p][64 * e:64 * (e + 1),
                                      c * 512:(c + 1) * 512],
                                po[0:64, :], rb[:],
                            )
                        if p == 1:
                            ph3_q.append(c)
                    elif ph3_q:
                        emit_phase3(ph3_q.pop(0))

                for p in range(2):
                    for c in range(NI):
                        po_e = psA.tile([65, 512], f32, tag="poe", name="poe")
                        po_o = psA.tile([65, 512], f32, tag="poo", name="poo")
                        for J in range(4 * c + 4):
                            pump(pre_q, (p, c, J), _PACE)
                            diag = (J // 4 == c)
                            s = J % 4
                            w = 512 - 128 * s if diag else 512
                            off = 128 * s if diag else 0
                            sc = psS.tile([128, 1024], f32, tag="sc", name="sc")
                            for e in range(2):
                                nc.tensor.matmul(
                                    sc[:, (512 - w) if e == 0 else 512:
                                       512 if e == 0 else 512 + w],
                                    kh[p][64 * e:64 * (e + 1),
                                          J * 128:(J + 1) * 128],
                                    qp[p][64 * e:64 * (e + 1),
                                          c * 512 + off:(c + 1) * 512],
                                    start=True, stop=not diag,
                                    skip_group_check=True,
                                )
                            if diag:
                                # causal: accumulate -240*(j-i) on diag strip
                                nc.tensor.matmul(
                                    sc[:, 512 - w:512 - w + 128],
                                    tri_s[:], mskb_s[:],
                                    start=False, stop=True,
                                    skip_group_check=True,
                                )
                                nc.tensor.matmul(
                                    sc[:, 512:512 + 128],
                                    tri_s[:], mskb_s[:],
                                    start=False, stop=True,
                                    skip_group_check=True,
                                )
                            et = sb_e.tile([128, 1024], dt, tag="et", name="et")
                            nc.scalar.activation(
                                et[:, 512 - w:512 - w + 2 * w],
                                sc[:, 512 - w:512 - w + 2 * w],
                                mybir.ActivationFunctionType.Exp,
                                bias=ebias[:], scale=EXP_SCALE,
                            )
                            pump(post_q, (p, c, J), _PACE)
                            if len(pending) >= _AVDEPTH:
                                flush()
                            pending.append((p, c, J, et, w, po_e, po_o))
                while pre_q:
                    pre_q.pop(0)[1]()
                while post_q:
                    post_q.pop(0)[1]()
                while pending:
                    flush()
                while ph3_q:
                    emit_phase3(ph3_q.pop(0))

        if n_iters > 1 and os.environ.get("BASS_MHA_UNROLL", "0") == "1":
            for _ in range(n_iters):
                body()
        elif n_iters > 1:
            with tc.For_i(0, n_iters, 1):
                body()
        else:
            body()

    nc.compile()
    return nc


_CACHE = {}


def _get_program(dt_name: str, n_iters: int = 1):
    key = (dt_name, n_iters, _OUT_DT, _AVDEPTH, _PACE)
    if key not in _CACHE:
        _CACHE[key] = _build(dt_name, n_iters)
    return _CACHE[key]


def _np_dt(dt_name: str):
    if dt_name == "bf16":
        import ml_dtypes
        return ml_dtypes.bfloat16
    return np.float32


def make_in_maps(q, k, v, w_q, w_k, w_v, w_o, dt_name: str):
    """Build the 8 per-core input dicts (host-side shard + transpose)."""
    ndt = _np_dt(dt_name)
    tri = np.triu(np.ones((128, 128), np.float32)).astype(ndt)
    mskb = (-240.0 * np.tril(np.ones((128, 128), np.float32), -1)).astype(ndt)
    in_maps = []
    for b in range(B):
        xqT = np.ascontiguousarray(q[b].T).astype(ndt)
        xkT = np.ascontiguousarray(k[b].T).astype(ndt)
        xvT = np.ascontiguousarray(v[b].T).astype(ndt)
        for g in range(N_GROUPS):
            r0 = GD * g
            in_maps.append({
                "xqT": xqT,
                "xkT": xkT,
                "xvT": xvT,
                "wqT": np.ascontiguousarray(w_q[r0:r0 + GD, :].T).astype(ndt),
                "wkT": np.ascontiguousarray(w_k[r0:r0 + GD, :].T).astype(ndt),
                "wvT": np.ascontiguousarray(w_v[r0:r0 + GD, :].T).astype(ndt),
                "woT": np.ascontiguousarray(w_o[:, r0:r0 + GD].T).astype(ndt),
                "tri": tri,
                "mskb": mskb,
            })
    return in_maps


def kernel(q, k, v, w_q, w_k, w_v, w_o):
    from concourse.bass_utils import run_bass_kernel_spmd

    dt_name = _DT
    nc = _get_program(dt_name)
    in_maps = make_in_maps(q, k, v, w_q, w_k, w_v, w_o, dt_name)
    res = run_bass_kernel_spmd(nc, in_maps, core_ids=list(range(N_CORES)))
    parts = [np.asarray(res.results[i]["outT"], dtype=np.float32)
             for i in range(N_CORES)]
    out = np.empty((B, N, D_MODEL), np.float32)
    for b in range(B):
        acc = parts[N_GROUPS * b]
        for g in range(1, N_GROUPS):
            acc = acc + parts[N_GROUPS * b + g]
        out[b] = acc.T
    return out


# revision 17
# speedup vs baseline: 1.0430x; 1.0010x over previous
"""Multi-head causal attention on 8 Trainium2 NeuronCores.

Sharding: core = (batch b in {0,1}) x (head-group g in {0..3}); each core
computes 4 of the 16 heads for one batch element and returns a partial
(d_model, n) output (its heads' contribution to the final projection).
Host sums the 4 partials per batch (w_o row-parallel reduce), transposes,
and stacks.

Single woven stream: the attention j-stream is ScalarE-paced (one exp
per step covers both heads of a pair, ~1.1us) while its own matmuls
need only ~0.6us of PE — so ALL projection work that the baseline ran
as a separate PE-only phase is fed into that slack through two
deadline-driven filler queues:

  DMA:       ONE batched 3D-AP DMA per tensor (x inputs on the sync
             ring in order xk, xv, xq so the last-arriving input gates
             only qp[0]; weights on the scalar ring) — 9 dma_starts per
             iteration instead of 55, amortizing the ~2us per-DMA
             completion latency.
  startup:   kh[0] i0..3, va[0..1], qp[0] i0 (k-outer per i-chunk, 2
             rotating psX banks), then the (p=0, c, J) stream begins.
  pre_q:     qp[0] i1..3, kh[1]/qp[1] i-chunks — pumped BEFORE the
             scores of their deadline step (kh[1] gets the earliest
             deadlines: it depends only on xk, so in a cold start it
             fills the PE while the stream waits on xq).
  post_q:    remaining va[nt] m64 V-projection chunks — needed only by
             the AV flush _AVDEPTH steps later, so pumped AFTER each
             step's exp.
  phase 3:   output projection per i-chunk interleaved into the p=1
             stream, sharing the psX pool.
  PSUM:      scores 2x2 banks + AV accumulators 2 + shared psX 2 = 8.

This keeps the PE continuously busy (no HAM re-throttle) instead of
alternating a PE-only projection phase with an exp-gated attention
phase.  Measured (loop-amortized, device-resident inputs): baseline
247.3us/iter -> 242us/iter; the stream itself is PE-dense in the
timeline sim (97% busy between startup and tail).

Attention math per step (p, c, J) is unchanged from the baseline:
scores for BOTH heads of a pair via concurrent row-tiled K=64 matmuls
into one [128,1024] psum; causal mask accumulated on the diagonal strip
by a tri x mskb matmul; ONE exp activation per step covering both heads;
AV via [V_h | 1]-augmented matmuls (denominator for free) software-
pipelined _AVDEPTH steps behind scores; normalize (reciprocal+broadcast+
mul) per pass end on DVE/Pool.
"""

import math
import os

import numpy as np

H = 16
D_MODEL = 1024
D_K = 64
N = 2048
B = 2
N_CORES = 8
N_GROUPS = 4          # head groups (tensor parallel)
HPC = H // N_GROUPS   # heads per core = 4
GD = HPC * D_K        # group output dim = 256
EXP_SCALE = 1.0 / math.sqrt(D_K)
EXP_BIAS = -5.0

_DT = os.environ.get("BASS_MHA_DT", "bf16")
_OUT_DT = os.environ.get("BASS_MHA_OUTDT", "bf16")
_AVDEPTH = int(os.environ.get("BASS_MHA_AVD", "2"))
_PACE = int(os.environ.get("BASS_MHA_PACE", "0"))

KC = D_MODEL // 128   # 8 contraction chunks
NI = N // 512         # 4 i-chunks of 512
NJ = N // 128         # 16 j-chunks of 128


def _build(dt_name: str, n_iters: int = 1):
    """Emit and compile the single-core SPMD program. Returns compiled nc."""
    import concourse.bacc as bacc
    import concourse.mybir as mybir
    import concourse.tile as tile
    from concourse.ap import AP

    dt = {"bf16": mybir.dt.bfloat16, "f32r": mybir.dt.float32r}[dt_name]
    f32 = mybir.dt.float32
    odt = {"bf16": mybir.dt.bfloat16, "f32": f32}[_OUT_DT]

    nc = bacc.Bacc("TRN2", num_devices=N_CORES)

    xqT = nc.dram_tensor("xqT", [D_MODEL, N], dt, kind="ExternalInput").ap()
    xkT = nc.dram_tensor("xkT", [D_MODEL, N], dt, kind="ExternalInput").ap()
    xvT = nc.dram_tensor("xvT", [D_MODEL, N], dt, kind="ExternalInput").ap()
    wqT = nc.dram_tensor("wqT", [D_MODEL, GD], dt, kind="ExternalInput").ap()
    wkT = nc.dram_tensor("wkT", [D_MODEL, GD], dt, kind="ExternalInput").ap()
    wvT = nc.dram_tensor("wvT", [D_MODEL, GD], dt, kind="ExternalInput").ap()
    woT = nc.dram_tensor("woT", [GD, D_MODEL], dt, kind="ExternalInput").ap()
    tri = nc.dram_tensor("tri", [128, 128], dt, kind="ExternalInput").ap()
    mskb = nc.dram_tensor("mskb", [128, 128], dt, kind="ExternalInput").ap()
    outT = nc.dram_tensor("outT", [D_MODEL, N], odt, kind="ExternalOutput").ap()

    xq_b = xqT.rearrange("(kc p) i -> p kc i", p=128)
    xk_b = xkT.rearrange("(kc p) i -> p kc i", p=128)
    xv_b = xvT.rearrange("(kc p) i -> p kc i", p=128)
    wq_b = wqT.rearrange("(kc p) m -> p kc m", p=128)
    wk_b = wkT.rearrange("(kc p) m -> p kc m", p=128)
    wv_b = wvT.rearrange("(kc p) m -> p kc m", p=128)
    wo_b = woT.rearrange("(oc p) m -> p oc m", p=128)

    from contextlib import ExitStack

    with tile.TileContext(nc) as tc, ExitStack() as ctx:
        sb_w = ctx.enter_context(tc.tile_pool(name="weights", bufs=1))
        sb_x = ctx.enter_context(tc.tile_pool(name="xin", bufs=1))
        sb_p = ctx.enter_context(tc.tile_pool(name="persist", bufs=1))
        sb_e = ctx.enter_context(tc.tile_pool(name="expw", bufs=4))
        sb_o = ctx.enter_context(tc.tile_pool(name="outw", bufs=2))

        def setup():
            # ---- one-time: resident weights + va ones columns ----
            wqb = sb_w.tile([128, KC * GD], dt, tag="wqb", name="wqb")
            wkb = sb_w.tile([128, KC * GD], dt, tag="wkb", name="wkb")
            wvb = sb_w.tile([128, KC * GD], dt, tag="wvb", name="wvb")
            wob = sb_w.tile([128, 2 * D_MODEL], dt, tag="wob", name="wob")
            tri_s = sb_w.tile([128, 128], dt, tag="tri", name="tri")
            mskb_s = sb_w.tile([128, 128], dt, tag="mskb", name="mskb")
            ebias = sb_w.tile([128, 1], f32, tag="ebias", name="ebias")
            nc.vector.memset(ebias[:], EXP_BIAS)
            # one batched DMA per tensor: [p, (kc m)] gather of all chunks
            nc.scalar.dma_start(
                wkb[:].rearrange("p (kc m) -> p kc m", kc=KC), wk_b)
            nc.scalar.dma_start(
                wvb[:].rearrange("p (kc m) -> p kc m", kc=KC), wv_b)
            nc.scalar.dma_start(
                wqb[:].rearrange("p (kc m) -> p kc m", kc=KC), wq_b)
            nc.scalar.dma_start(
                wob[:].rearrange("p (oc m) -> p oc m", oc=2), wo_b)
            nc.scalar.dma_start(tri_s[:], tri[:])
            nc.scalar.dma_start(mskb_s[:], mskb[:])
            wk_s = [wkb[:, k * GD:(k + 1) * GD] for k in range(KC)]
            wq_s = [wqb[:, k * GD:(k + 1) * GD] for k in range(KC)]
            wv_s = [wvb[:, k * GD:(k + 1) * GD] for k in range(KC)]
            wo_s = [wob[:, o * D_MODEL:(o + 1) * D_MODEL] for o in range(2)]

            # V natural augmented per j-chunk: [128 j, (V_h | 1) x 4].
            # The ones columns are written once here and never touched by
            # the per-iteration evacs (which write only cols 0..63 of each
            # head section), so they persist across loop iterations.
            va = [sb_p.tile([128, 4 * 65], dt, tag=f"va{nt}", name=f"va{nt}") for nt in range(NJ)]
            for nt in range(NJ):
                for h in range(HPC):
                    nc.vector.memset(va[nt][:, h * 65 + 64:h * 65 + 65], 1.0)
            return (wk_s, wq_s, wv_s, wo_s, tri_s, mskb_s, ebias, va)

        hands = setup()

        def body():
            wk_s, wq_s, wv_s, wo_s, tri_s, mskb_s, ebias, va = hands

            # ---- per-chunk x inputs (sync ring, in consumption order) ----
            xkb = sb_x.tile([128, KC * N], dt, tag="xkb", name="xkb")
            xqb = sb_x.tile([128, KC * N], dt, tag="xqb", name="xqb")
            xvb = sb_x.tile([128, KC * N], dt, tag="xvb", name="xvb")
            nc.sync.dma_start(
                xkb[:].rearrange("p (kc i) -> p kc i", kc=KC), xk_b)
            nc.sync.dma_start(
                xvb[:].rearrange("p (kc i) -> p kc i", kc=KC), xv_b)
            nc.sync.dma_start(
                xqb[:].rearrange("p (kc i) -> p kc i", kc=KC), xq_b)
            xk_c = [xkb[:, k * N:(k + 1) * N] for k in range(KC)]
            xq_c = [xqb[:, k * N:(k + 1) * N] for k in range(KC)]
            xv_c = [xvb[:, k * N:(k + 1) * N] for k in range(KC)]

            # ---- persistent intermediates ----
            # transposed pair tiles: [(h_even d64 | h_odd d64), n]
            kh = [sb_p.tile([128, N], dt, tag=f"kh{m}", name=f"kh{m}") for m in range(2)]
            qp = [sb_p.tile([128, N], dt, tag=f"qp{m}", name=f"qp{m}") for m in range(2)]
            # normalized O^T per pair: [(h_even d64 | h_odd d64), n]
            ot = [sb_p.tile([128, N], dt, tag=f"ot{p}", name=f"ot{p}") for p in range(2)]

            with tc.tile_pool(name="psS", bufs=2, space="PSUM") as psS, \
                 tc.tile_pool(name="psA", bufs=1, space="PSUM") as psA, \
                 tc.tile_pool(name="psX", bufs=2, space="PSUM") as psX:

                # ---------- filler units (all use the shared psX pool) ----
                def proj_chunk(w_s, x_c, dst, m, i):
                    # one i-chunk of a transposed projection m-pass
                    px = psX.tile([128, 512], f32, tag="px", name="px")
                    for k in range(KC):
                        nc.tensor.matmul(
                            px[:],
                            w_s[k][:, m * 128:(m + 1) * 128],
                            x_c[k][:, i * 512:(i + 1) * 512],
                            start=(k == 0), stop=(k == KC - 1),
                        )
                    nc.vector.tensor_copy(
                        dst[m][:, i * 512:(i + 1) * 512], px[:])

                def v_chunk(nt):
                    # natural-layout col-tiled m64 V projection for j-chunk nt
                    px = psX.tile([128, 512], f32, tag="px", name="px")
                    for k in range(KC):
                        for u in range(2):
                            nc.tensor.matmul(
                                px[64 * u:64 * (u + 1), 0:256],
                                xv_c[k][:, nt * 128 + 64 * u:
                                        nt * 128 + 64 * (u + 1)],
                                wv_s[k][:],
                                start=(k == 0), stop=(k == KC - 1),
                                tile_position=(0, 64 * u),
                                skip_group_check=True,
                            )
                    for h in range(HPC):
                        nc.vector.tensor_copy(
                            va[nt][:, h * 65:h * 65 + 64],
                            px[:, h * 64:(h + 1) * 64])

                def emit_phase3(c):
                    # output projection for i-chunk c (both pairs' ot ready)
                    us = sb_o.tile([128, 8 * 512], odt, tag="us", name="us")
                    for ms in range(8):
                        pu = psX.tile([128, 512], f32, tag="px", name="px")
                        for p in range(2):
                            nc.tensor.matmul(
                                pu[:],
                                wo_s[p][:, ms * 128:(ms + 1) * 128],
                                ot[p][:, c * 512:(c + 1) * 512],
                                start=(p == 0), stop=(p == 1),
                            )
                        nc.vector.tensor_copy(
                            us[:, ms * 512:(ms + 1) * 512], pu[:])
                    nc.scalar.dma_start(
                        AP(outT.tensor, c * 512,
                           [[N, 128], [128 * N, 8], [1, 512]]),
                        us[:],
                    )

                # ---------- filler queues ----------
                # (deadline (p,c,J), emit_fn); deadlines are hand-spread so
                # roughly one unit lands per step, always at or before the
                # unit's hard consumption deadline.
                def _pj(w_s, x_c, dst, m, i):
                    return lambda: proj_chunk(w_s, x_c, dst, m, i)

                # pre_q units must land BEFORE the scores of their deadline
                # step (they produce kh/qp operands of that step).  kh[1]
                # units depend only on xk (first to arrive) so they get the
                # earliest deadlines — in a cold start they fill the PE
                # while the stream waits on xq; in the loop steady state
                # the stream paces them into its slack.
                pre_q = [
                    ((0, 0, 0), _pj(wk_s, xk_c, kh, 1, 0)),
                    ((0, 0, 2), _pj(wk_s, xk_c, kh, 1, 1)),
                    ((0, 1, 0), _pj(wq_s, xq_c, qp, 0, 1)),
                    ((0, 1, 2), _pj(wk_s, xk_c, kh, 1, 2)),
                    ((0, 1, 6), _pj(wk_s, xk_c, kh, 1, 3)),
                    ((0, 2, 0), _pj(wq_s, xq_c, qp, 0, 2)),
                    ((0, 2, 4), _pj(wq_s, xq_c, qp, 1, 0)),
                    ((0, 3, 0), _pj(wq_s, xq_c, qp, 0, 3)),
                    ((0, 3, 6), _pj(wq_s, xq_c, qp, 1, 1)),
                    ((1, 1, 0), _pj(wq_s, xq_c, qp, 1, 2)),
                    ((1, 2, 0), _pj(wq_s, xq_c, qp, 1, 3)),
                ]
                # post_q units are needed only by the AV flush _AVDEPTH
                # steps later, so they are pumped after the step's exp.
                # va[nt] is first consumed by the flush of step
                # (0, c_min(nt), nt); deadlines sit at or before that.
                post_q = [
                    ((0, 0, 0), lambda: v_chunk(2)),
                    ((0, 0, 1), lambda: v_chunk(3)),
                    ((0, 0, 2), lambda: v_chunk(4)),
                    ((0, 0, 3), lambda: v_chunk(5)),
                    ((0, 1, 1), lambda: v_chunk(6)),
                    ((0, 1, 3), lambda: v_chunk(7)),
                    ((0, 1, 5), lambda: v_chunk(8)),
                    ((0, 1, 7), lambda: v_chunk(9)),
                    ((0, 2, 2), lambda: v_chunk(10)),
                    ((0, 2, 5), lambda: v_chunk(11)),
                    ((0, 2, 8), lambda: v_chunk(12)),
                    ((0, 2, 11), lambda: v_chunk(13)),
                    ((0, 3, 4), lambda: v_chunk(14)),
                    ((0, 3, 9), lambda: v_chunk(15)),
                ]

                def pump(q, key, extra=0):
                    # emit all units due at `key`, plus up to `extra` more
                    n = 0
                    while q and (q[0][0] <= key or n < extra):
                        if q[0][0] > key:
                            n += 1
                        q.pop(0)[1]()

                # ---------- startup ----------
                for i in range(NI):
                    proj_chunk(wk_s, xk_c, kh, 0, i)
                v_chunk(0)
                v_chunk(1)
                proj_chunk(wq_s, xq_c, qp, 0, 0)

                # ---------- attention stream ----------
                pending = []    # [(p, c, J, et, w, po_e, po_o)]
                ph3_q = []      # i-chunks whose output projection is due

                def flush():
                    if not pending:
                        return
                    p, c, J, et, w, po_e, po_o = pending.pop(0)
                    off = 512 - w
                    for e, po in ((0, po_e), (1, po_o)):
                        h = 2 * p + e
                        lo = (512 - w) if e == 0 else 512
                        nc.tensor.matmul(
                            po[:, off:512],
                            va[J][:, h * 65:h * 65 + 65],
                            et[:, lo:lo + w],
                            start=(J == 0), stop=(J == 4 * c + 3),
                            skip_group_check=True,
                        )
                    if J == 4 * c + 3:
                        # normalize chunk c for both heads of pair p
                        for e, po in ((0, po_e), (1, po_o)):
                            rec = sb_o.tile([1, 512], f32, tag="rec", name="rec")
                            nc.vector.reciprocal(rec[:], po[64:65, :])
                            rb = sb_o.tile([64, 512], f32, tag="rb", name="rb")
                            nc.gpsimd.partition_broadcast(rb[:], rec[0:1, :])
                            nc.vector.tensor_mul(
                                ot[p][64 * e:64 * (e + 1),
                                      c * 512:(c + 1) * 512],
                                po[0:64, :], rb[:],
                            )
                        if p == 1:
                            ph3_q.append(c)
                    elif ph3_q:
                        emit_phase3(ph3_q.pop(0))

                for p in range(2):
                    for c in range(NI):
                        po_e = psA.tile([65, 512], f32, tag="poe", name="poe")
                        po_o = psA.tile([65, 512], f32, tag="poo", name="poo")
                        for J in range(4 * c + 4):
                            pump(pre_q, (p, c, J), _PACE)
                            diag = (J // 4 == c)
                            s = J % 4
                            w = 512 - 128 * s if diag else 512
                            off = 128 * s if diag else 0
                            sc = psS.tile([128, 1024], f32, tag="sc", name="sc")
                            for e in range(2):
                                nc.tensor.matmul(
                                    sc[:, (512 - w) if e == 0 else 512:
                                       512 if e == 0 else 512 + w],
                                    kh[p][64 * e:64 * (e + 1),
                                          J * 128:(J + 1) * 128],
                                    qp[p][64 * e:64 * (e + 1),
                                          c * 512 + off:(c + 1) * 512],
                                    start=True, stop=not diag,
                                    skip_group_check=True,
                                )
                            if diag:
                                # causal: accumulate -240*(j-i) on diag strip
                                nc.tensor.matmul(
                                    sc[:, 512 - w:512 - w + 128],
                                    tri_s[:], mskb_s[:],
                                    start=False, stop=True,
                                    skip_group_check=True,
                                )
                                nc.tensor.matmul(
                                    sc[:, 512:512 + 128],
                                    tri_s[:], mskb_s[:],
                                    start=False, stop=True,
                                    skip_group_check=True,
                                )
                            et = sb_e.tile([128, 1024], dt, tag="et", name="et")
                            nc.scalar.activation(
                                et[:, 512 - w:512 - w + 2 * w],
                                sc[:, 512 - w:512 - w + 2 * w],
                                mybir.ActivationFunctionType.Exp,
                                bias=ebias[:], scale=EXP_SCALE,
                            )
                            pump(post_q, (p, c, J), _PACE)
                            if len(pending) >= _AVDEPTH:
                                flush()
                            pending.append((p, c, J, et, w, po_e, po_o))
                while pre_q:
                    pre_q.pop(0)[1]()
                while post_q:
                    post_q.pop(0)[1]()
                while pending:
                    flush()
                while ph3_q:
                    emit_phase3(ph3_q.pop(0))

        if n_iters > 1 and os.environ.get("BASS_MHA_UNROLL", "0") == "1":
            for _ in range(n_iters):
                body()
        elif n_iters > 1:
            with tc.For_i(0, n_iters, 1):
                body()
        else:
            body()

    nc.compile()
    return nc


_CACHE = {}


def _get_program(dt_name: str, n_iters: int = 1):
    key = (dt_name, n_iters, _OUT_DT, _AVDEPTH, _PACE)
    if key not in _CACHE:
        _CACHE[key] = _build(dt_name, n_iters)
    return _CACHE[key]


def _np_dt(dt_name: str):
    if dt_name == "bf16":
        import ml_dtypes
        return ml_dtypes.bfloat16
    return np.float32


def make_in_maps(q, k, v, w_q, w_k, w_v, w_o, dt_name: str):
    """Build the 8 per-core input dicts (host-side shard + transpose)."""
    ndt = _np_dt(dt_name)
    tri = np.triu(np.ones((128, 128), np.float32)).astype(ndt)
    mskb = (-240.0 * np.tril(np.ones((128, 128), np.float32), -1)).astype(ndt)
    in_maps = []
    for b in range(B):
        xqT = np.ascontiguousarray(q[b].T).astype(ndt)
        xkT = np.ascontiguousarray(k[b].T).astype(ndt)
        xvT = np.ascontiguousarray(v[b].T).astype(ndt)
        for g in range(N_GROUPS):
            r0 = GD * g
            in_maps.append({
                "xqT": xqT,
                "xkT": xkT,
                "xvT": xvT,
                "wqT": np.ascontiguousarray(w_q[r0:r0 + GD, :].T).astype(ndt),
                "wkT": np.ascontiguousarray(w_k[r0:r0 + GD, :].T).astype(ndt),
                "wvT": np.ascontiguousarray(w_v[r0:r0 + GD, :].T).astype(ndt),
                "woT": np.ascontiguousarray(w_o[:, r0:r0 + GD].T).astype(ndt),
                "tri": tri,
                "mskb": mskb,
            })
    return in_maps


def kernel(q, k, v, w_q, w_k, w_v, w_o):
    from concourse.bass_utils import run_bass_kernel_spmd

    dt_name = _DT
    nc = _get_program(dt_name)
    in_maps = make_in_maps(q, k, v, w_q, w_k, w_v, w_o, dt_name)
    res = run_bass_kernel_spmd(nc, in_maps, core_ids=list(range(N_CORES)))
    parts = [np.asarray(res.results[i]["outT"], dtype=np.float32)
             for i in range(N_CORES)]
    out = np.empty((B, N, D_MODEL), np.float32)
    for b in range(B):
        acc = parts[N_GROUPS * b]
        for g in range(1, N_GROUPS):
            acc = acc + parts[N_GROUPS * b + g]
        out[b] = acc.T
    return out


# revision 18
# speedup vs baseline: 1.0500x; 1.0067x over previous
"""Multi-head causal attention on 8 Trainium2 NeuronCores.

Sharding: core = (batch b in {0,1}) x (head-group g in {0..3}); each core
computes 4 of the 16 heads for one batch element and returns a partial
(d_model, n) output (its heads' contribution to the final projection).
Host sums the 4 partials per batch (w_o row-parallel reduce), transposes,
and stacks.

Single woven stream: the attention j-stream is ScalarE-paced (one exp
per step covers both heads of a pair, ~1.1us) while its own matmuls
need only ~0.6us of PE — so ALL projection work that the baseline ran
as a separate PE-only phase is fed into that slack through two
deadline-driven filler queues:

  DMA:       ONE batched 3D-AP DMA per tensor (x inputs on the sync
             ring in order xk, xv, xq so the last-arriving input gates
             only qp[0]; weights on the scalar ring) — 9 dma_starts per
             iteration instead of 55, amortizing the ~2us per-DMA
             completion latency.
  startup:   kh[0] i0..3, va[0..1], qp[0] i0 (k-outer per i-chunk, 2
             rotating psX banks), then the (p=0, c, J) stream begins.
  pre_q:     qp[0] i1..3, kh[1]/qp[1] i-chunks — pumped BEFORE the
             scores of their deadline step (kh[1] gets the earliest
             deadlines: it depends only on xk, so in a cold start it
             fills the PE while the stream waits on xq).
  post_q:    remaining va[nt] m64 V-projection chunks — needed only by
             the AV flush _AVDEPTH steps later, so pumped AFTER each
             step's exp.
  phase 3:   output projection per i-chunk interleaved into the p=1
             stream, sharing the psX pool.
  PSUM:      scores 2x2 banks + AV accumulators 2 + shared psX 2 = 8.

This keeps the PE continuously busy (no HAM re-throttle) instead of
alternating a PE-only projection phase with an exp-gated attention
phase.  Measured (loop-amortized, device-resident inputs): baseline
247.3us/iter -> 242us/iter; the stream itself is PE-dense in the
timeline sim (97% busy between startup and tail).

Attention math per step (p, c, J) is unchanged from the baseline:
scores for BOTH heads of a pair via concurrent row-tiled K=64 matmuls
into one [128,1024] psum; causal mask accumulated on the diagonal strip
by a tri x mskb matmul; ONE exp activation per step covering both heads;
AV via [V_h | 1]-augmented matmuls (denominator for free) software-
pipelined _AVDEPTH steps behind scores; normalize (reciprocal+broadcast+
mul) per pass end on DVE/Pool.
"""

import math
import os

import numpy as np

H = 16
D_MODEL = 1024
D_K = 64
N = 2048
B = 2
N_CORES = 8
N_GROUPS = 4          # head groups (tensor parallel)
HPC = H // N_GROUPS   # heads per core = 4
GD = HPC * D_K        # group output dim = 256
EXP_SCALE = 1.0 / math.sqrt(D_K)
EXP_BIAS = -5.0

_DT = os.environ.get("BASS_MHA_DT", "bf16")
_OUT_DT = os.environ.get("BASS_MHA_OUTDT", "bf16")
_AVDEPTH = int(os.environ.get("BASS_MHA_AVD", "2"))
_PACE = int(os.environ.get("BASS_MHA_PACE", "0"))

KC = D_MODEL // 128   # 8 contraction chunks
NI = N // 512         # 4 i-chunks of 512
NJ = N // 128         # 16 j-chunks of 128


def _build(dt_name: str, n_iters: int = 1):
    """Emit and compile the single-core SPMD program. Returns compiled nc."""
    import concourse.bacc as bacc
    import concourse.mybir as mybir
    import concourse.tile as tile
    from concourse.ap import AP

    dt = {"bf16": mybir.dt.bfloat16, "f32r": mybir.dt.float32r}[dt_name]
    f32 = mybir.dt.float32
    odt = {"bf16": mybir.dt.bfloat16, "f32": f32}[_OUT_DT]

    nc = bacc.Bacc("TRN2", num_devices=N_CORES)

    xqT = nc.dram_tensor("xqT", [D_MODEL, N], dt, kind="ExternalInput").ap()
    xkT = nc.dram_tensor("xkT", [D_MODEL, N], dt, kind="ExternalInput").ap()
    xvT = nc.dram_tensor("xvT", [D_MODEL, N], dt, kind="ExternalInput").ap()
    wqT = nc.dram_tensor("wqT", [D_MODEL, GD], dt, kind="ExternalInput").ap()
    wkT = nc.dram_tensor("wkT", [D_MODEL, GD], dt, kind="ExternalInput").ap()
    wvT = nc.dram_tensor("wvT", [D_MODEL, GD], dt, kind="ExternalInput").ap()
    woT = nc.dram_tensor("woT", [GD, D_MODEL], dt, kind="ExternalInput").ap()
    tri = nc.dram_tensor("tri", [128, 128], dt, kind="ExternalInput").ap()
    mskb = nc.dram_tensor("mskb", [128, 128], dt, kind="ExternalInput").ap()
    outT = nc.dram_tensor("outT", [D_MODEL, N], odt, kind="ExternalOutput").ap()

    xq_b = xqT.rearrange("(kc p) i -> p kc i", p=128)
    xk_b = xkT.rearrange("(kc p) i -> p kc i", p=128)
    xv_b = xvT.rearrange("(kc p) i -> p kc i", p=128)
    wq_b = wqT.rearrange("(kc p) m -> p kc m", p=128)
    wk_b = wkT.rearrange("(kc p) m -> p kc m", p=128)
    wv_b = wvT.rearrange("(kc p) m -> p kc m", p=128)
    wo_b = woT.rearrange("(oc p) m -> p oc m", p=128)

    from contextlib import ExitStack

    with tile.TileContext(nc) as tc, ExitStack() as ctx:
        sb_w = ctx.enter_context(tc.tile_pool(name="weights", bufs=1))
        sb_x = ctx.enter_context(tc.tile_pool(name="xin", bufs=1))
        sb_p = ctx.enter_context(tc.tile_pool(name="persist", bufs=1))
        sb_e = ctx.enter_context(tc.tile_pool(name="expw", bufs=4))
        sb_o = ctx.enter_context(tc.tile_pool(name="outw", bufs=2))

        def setup():
            # ---- one-time: resident weights + va ones columns ----
            wqb = sb_w.tile([128, KC * GD], dt, tag="wqb", name="wqb")
            wkb = sb_w.tile([128, KC * GD], dt, tag="wkb", name="wkb")
            wvb = sb_w.tile([128, KC * GD], dt, tag="wvb", name="wvb")
            wob = sb_w.tile([128, 2 * D_MODEL], dt, tag="wob", name="wob")
            tri_s = sb_w.tile([128, 128], dt, tag="tri", name="tri")
            mskb_s = sb_w.tile([128, 128], dt, tag="mskb", name="mskb")
            ebias = sb_w.tile([128, 1], f32, tag="ebias", name="ebias")
            nc.vector.memset(ebias[:], EXP_BIAS)
            # one batched DMA per tensor: [p, (kc m)] gather of all chunks
            nc.scalar.dma_start(
                wkb[:].rearrange("p (kc m) -> p kc m", kc=KC), wk_b)
            nc.scalar.dma_start(
                wvb[:].rearrange("p (kc m) -> p kc m", kc=KC), wv_b)
            nc.scalar.dma_start(
                wqb[:].rearrange("p (kc m) -> p kc m", kc=KC), wq_b)
            nc.scalar.dma_start(
                wob[:].rearrange("p (oc m) -> p oc m", oc=2), wo_b)
            nc.scalar.dma_start(tri_s[:], tri[:])
            nc.scalar.dma_start(mskb_s[:], mskb[:])
            wk_s = [wkb[:, k * GD:(k + 1) * GD] for k in range(KC)]
            wq_s = [wqb[:, k * GD:(k + 1) * GD] for k in range(KC)]
            wv_s = [wvb[:, k * GD:(k + 1) * GD] for k in range(KC)]
            wo_s = [wob[:, o * D_MODEL:(o + 1) * D_MODEL] for o in range(2)]

            # V natural augmented per j-chunk: [128 j, (V_h | 1) x 4].
            # The ones columns are written once here and never touched by
            # the per-iteration evacs (which write only cols 0..63 of each
            # head section), so they persist across loop iterations.
            va = [sb_p.tile([128, 4 * 65], dt, tag=f"va{nt}", name=f"va{nt}") for nt in range(NJ)]
            for nt in range(NJ):
                for h in range(HPC):
                    nc.vector.memset(va[nt][:, h * 65 + 64:h * 65 + 65], 1.0)
            return (wk_s, wq_s, wv_s, wo_s, tri_s, mskb_s, ebias, va)

        hands = setup()

        def body():
            wk_s, wq_s, wv_s, wo_s, tri_s, mskb_s, ebias, va = hands

            # ---- per-chunk x inputs (sync ring, in consumption order) ----
            xkb = sb_x.tile([128, KC * N], dt, tag="xkb", name="xkb")
            xqb = sb_x.tile([128, KC * N], dt, tag="xqb", name="xqb")
            xvb = sb_x.tile([128, KC * N], dt, tag="xvb", name="xvb")
            # two half-tensor DMAs per input: keeps the dma_start count low
            # while letting the first projection matmuls start when the
            # first half lands (arrival-driven cold start)
            H2 = KC // 2
            for xb, x_b in ((xkb, xk_b), (xvb, xv_b), (xqb, xq_b)):
                for h2 in range(2):
                    nc.sync.dma_start(
                        xb[:, h2 * H2 * N:(h2 + 1) * H2 * N].rearrange(
                            "p (kc i) -> p kc i", kc=H2),
                        x_b[:, h2 * H2:(h2 + 1) * H2])
            xk_c = [xkb[:, k * N:(k + 1) * N] for k in range(KC)]
            xq_c = [xqb[:, k * N:(k + 1) * N] for k in range(KC)]
            xv_c = [xvb[:, k * N:(k + 1) * N] for k in range(KC)]

            # ---- persistent intermediates ----
            # transposed pair tiles: [(h_even d64 | h_odd d64), n]
            kh = [sb_p.tile([128, N], dt, tag=f"kh{m}", name=f"kh{m}") for m in range(2)]
            qp = [sb_p.tile([128, N], dt, tag=f"qp{m}", name=f"qp{m}") for m in range(2)]
            # normalized O^T per pair: [(h_even d64 | h_odd d64), n]
            ot = [sb_p.tile([128, N], dt, tag=f"ot{p}", name=f"ot{p}") for p in range(2)]

            with tc.tile_pool(name="psS", bufs=2, space="PSUM") as psS, \
                 tc.tile_pool(name="psA", bufs=1, space="PSUM") as psA, \
                 tc.tile_pool(name="psX", bufs=2, space="PSUM") as psX:

                # ---------- filler units (all use the shared psX pool) ----
                def proj_chunk(w_s, x_c, dst, m, i):
                    # one i-chunk of a transposed projection m-pass
                    px = psX.tile([128, 512], f32, tag="px", name="px")
                    for k in range(KC):
                        nc.tensor.matmul(
                            px[:],
                            w_s[k][:, m * 128:(m + 1) * 128],
                            x_c[k][:, i * 512:(i + 1) * 512],
                            start=(k == 0), stop=(k == KC - 1),
                        )
                    nc.vector.tensor_copy(
                        dst[m][:, i * 512:(i + 1) * 512], px[:])

                def v_chunk(nt):
                    # natural-layout col-tiled m64 V projection for j-chunk nt
                    px = psX.tile([128, 512], f32, tag="px", name="px")
                    for k in range(KC):
                        for u in range(2):
                            nc.tensor.matmul(
                                px[64 * u:64 * (u + 1), 0:256],
                                xv_c[k][:, nt * 128 + 64 * u:
                                        nt * 128 + 64 * (u + 1)],
                                wv_s[k][:],
                                start=(k == 0), stop=(k == KC - 1),
                                tile_position=(0, 64 * u),
                                skip_group_check=True,
                            )
                    for h in range(HPC):
                        nc.vector.tensor_copy(
                            va[nt][:, h * 65:h * 65 + 64],
                            px[:, h * 64:(h + 1) * 64])

                def emit_phase3(c):
                    # output projection for i-chunk c (both pairs' ot ready)
                    us = sb_o.tile([128, 8 * 512], odt, tag="us", name="us")
                    for ms in range(8):
                        pu = psX.tile([128, 512], f32, tag="px", name="px")
                        for p in range(2):
                            nc.tensor.matmul(
                                pu[:],
                                wo_s[p][:, ms * 128:(ms + 1) * 128],
                                ot[p][:, c * 512:(c + 1) * 512],
                                start=(p == 0), stop=(p == 1),
                            )
                        nc.vector.tensor_copy(
                            us[:, ms * 512:(ms + 1) * 512], pu[:])
                    nc.scalar.dma_start(
                        AP(outT.tensor, c * 512,
                           [[N, 128], [128 * N, 8], [1, 512]]),
                        us[:],
                    )

                # ---------- filler queues ----------
                # (deadline (p,c,J), emit_fn); deadlines are hand-spread so
                # roughly one unit lands per step, always at or before the
                # unit's hard consumption deadline.
                def _pj(w_s, x_c, dst, m, i):
                    return lambda: proj_chunk(w_s, x_c, dst, m, i)

                # pre_q units must land BEFORE the scores of their deadline
                # step (they produce kh/qp operands of that step).  kh[1]
                # units depend only on xk (first to arrive) so they get the
                # earliest deadlines — in a cold start they fill the PE
                # while the stream waits on xq; in the loop steady state
                # the stream paces them into its slack.
                pre_q = [
                    ((0, 0, 0), _pj(wk_s, xk_c, kh, 1, 0)),
                    ((0, 0, 2), _pj(wk_s, xk_c, kh, 1, 1)),
                    ((0, 1, 0), _pj(wq_s, xq_c, qp, 0, 1)),
                    ((0, 1, 2), _pj(wk_s, xk_c, kh, 1, 2)),
                    ((0, 1, 6), _pj(wk_s, xk_c, kh, 1, 3)),
                    ((0, 2, 0), _pj(wq_s, xq_c, qp, 0, 2)),
                    ((0, 2, 4), _pj(wq_s, xq_c, qp, 1, 0)),
                    ((0, 3, 0), _pj(wq_s, xq_c, qp, 0, 3)),
                    ((0, 3, 6), _pj(wq_s, xq_c, qp, 1, 1)),
                    ((1, 1, 0), _pj(wq_s, xq_c, qp, 1, 2)),
                    ((1, 2, 0), _pj(wq_s, xq_c, qp, 1, 3)),
                ]
                # post_q units are needed only by the AV flush _AVDEPTH
                # steps later, so they are pumped after the step's exp.
                # va[nt] is first consumed by the flush of step
                # (0, c_min(nt), nt); deadlines sit at or before that.
                post_q = [
                    ((0, 0, 0), lambda: v_chunk(2)),
                    ((0, 0, 1), lambda: v_chunk(3)),
                    ((0, 0, 2), lambda: v_chunk(4)),
                    ((0, 0, 3), lambda: v_chunk(5)),
                    ((0, 1, 1), lambda: v_chunk(6)),
                    ((0, 1, 3), lambda: v_chunk(7)),
                    ((0, 1, 5), lambda: v_chunk(8)),
                    ((0, 1, 7), lambda: v_chunk(9)),
                    ((0, 2, 2), lambda: v_chunk(10)),
                    ((0, 2, 5), lambda: v_chunk(11)),
                    ((0, 2, 8), lambda: v_chunk(12)),
                    ((0, 2, 11), lambda: v_chunk(13)),
                    ((0, 3, 4), lambda: v_chunk(14)),
                    ((0, 3, 9), lambda: v_chunk(15)),
                ]

                def pump(q, key, extra=0):
                    # emit all units due at `key`, plus up to `extra` more
                    n = 0
                    while q and (q[0][0] <= key or n < extra):
                        if q[0][0] > key:
                            n += 1
                        q.pop(0)[1]()

                # ---------- startup ----------
                for i in range(NI):
                    proj_chunk(wk_s, xk_c, kh, 0, i)
                v_chunk(0)
                v_chunk(1)
                proj_chunk(wq_s, xq_c, qp, 0, 0)

                # ---------- attention stream ----------
                pending = []    # [(p, c, J, et, w, po_e, po_o)]
                ph3_q = []      # i-chunks whose output projection is due

                def flush():
                    if not pending:
                        return
                    p, c, J, et, w, po_e, po_o = pending.pop(0)
                    off = 512 - w
                    for e, po in ((0, po_e), (1, po_o)):
                        h = 2 * p + e
                        lo = (512 - w) if e == 0 else 512
                        nc.tensor.matmul(
                            po[:, off:512],
                            va[J][:, h * 65:h * 65 + 65],
                            et[:, lo:lo + w],
                            start=(J == 0), stop=(J == 4 * c + 3),
                            skip_group_check=True,
                        )
                    if J == 4 * c + 3:
                        # normalize chunk c for both heads of pair p
                        for e, po in ((0, po_e), (1, po_o)):
                            rec = sb_o.tile([1, 512], f32, tag="rec", name="rec")
                            nc.vector.reciprocal(rec[:], po[64:65, :])
                            rb = sb_o.tile([64, 512], f32, tag="rb", name="rb")
                            nc.gpsimd.partition_broadcast(rb[:], rec[0:1, :])
                            nc.vector.tensor_mul(
                                ot[p][64 * e:64 * (e + 1),
                                      c * 512:(c + 1) * 512],
                                po[0:64, :], rb[:],
                            )
                        if p == 1:
                            ph3_q.append(c)
                    elif ph3_q:
                        emit_phase3(ph3_q.pop(0))

                for p in range(2):
                    for c in range(NI):
                        po_e = psA.tile([65, 512], f32, tag="poe", name="poe")
                        po_o = psA.tile([65, 512], f32, tag="poo", name="poo")
                        for J in range(4 * c + 4):
                            pump(pre_q, (p, c, J), _PACE)
                            diag = (J // 4 == c)
                            s = J % 4
                            w = 512 - 128 * s if diag else 512
                            off = 128 * s if diag else 0
                            sc = psS.tile([128, 1024], f32, tag="sc", name="sc")
                            for e in range(2):
                                nc.tensor.matmul(
                                    sc[:, (512 - w) if e == 0 else 512:
                                       512 if e == 0 else 512 + w],
                                    kh[p][64 * e:64 * (e + 1),
                                          J * 128:(J + 1) * 128],
                                    qp[p][64 * e:64 * (e + 1),
                                          c * 512 + off:(c + 1) * 512],
                                    start=True, stop=not diag,
                                    skip_group_check=True,
                                )
                            if diag:
                                # causal: accumulate -240*(j-i) on diag strip
                                nc.tensor.matmul(
                                    sc[:, 512 - w:512 - w + 128],
                                    tri_s[:], mskb_s[:],
                                    start=False, stop=True,
                                    skip_group_check=True,
                                )
                                nc.tensor.matmul(
                                    sc[:, 512:512 + 128],
                                    tri_s[:], mskb_s[:],
                                    start=False, stop=True,
                                    skip_group_check=True,
                                )
                            et = sb_e.tile([128, 1024], dt, tag="et", name="et")
                            nc.scalar.activation(
                                et[:, 512 - w:512 - w + 2 * w],
                                sc[:, 512 - w:512 - w + 2 * w],
                                mybir.ActivationFunctionType.Exp,
                                bias=ebias[:], scale=EXP_SCALE,
                            )
                            pump(post_q, (p, c, J), _PACE)
                            if len(pending) >= _AVDEPTH:
                                flush()
                            pending.append((p, c, J, et, w, po_e, po_o))
                while pre_q:
                    pre_q.pop(0)[1]()
                while post_q:
                    post_q.pop(0)[1]()
                while pending:
                    flush()
                while ph3_q:
                    emit_phase3(ph3_q.pop(0))

        if n_iters > 1 and os.environ.get("BASS_MHA_UNROLL", "0") == "1":
            for _ in range(n_iters):
                body()
        elif n_iters > 1:
            with tc.For_i(0, n_iters, 1):
                body()
        else:
            body()

    nc.compile()
    return nc


_CACHE = {}


def _get_program(dt_name: str, n_iters: int = 1):
    key = (dt_name, n_iters, _OUT_DT, _AVDEPTH, _PACE)
    if key not in _CACHE:
        _CACHE[key] = _build(dt_name, n_iters)
    return _CACHE[key]


def _np_dt(dt_name: str):
    if dt_name == "bf16":
        import ml_dtypes
        return ml_dtypes.bfloat16
    return np.float32


def make_in_maps(q, k, v, w_q, w_k, w_v, w_o, dt_name: str):
    """Build the 8 per-core input dicts (host-side shard + transpose)."""
    ndt = _np_dt(dt_name)
    tri = np.triu(np.ones((128, 128), np.float32)).astype(ndt)
    mskb = (-240.0 * np.tril(np.ones((128, 128), np.float32), -1)).astype(ndt)
    in_maps = []
    for b in range(B):
        xqT = np.ascontiguousarray(q[b].T).astype(ndt)
        xkT = np.ascontiguousarray(k[b].T).astype(ndt)
        xvT = np.ascontiguousarray(v[b].T).astype(ndt)
        for g in range(N_GROUPS):
            r0 = GD * g
            in_maps.append({
                "xqT": xqT,
                "xkT": xkT,
                "xvT": xvT,
                "wqT": np.ascontiguousarray(w_q[r0:r0 + GD, :].T).astype(ndt),
                "wkT": np.ascontiguousarray(w_k[r0:r0 + GD, :].T).astype(ndt),
                "wvT": np.ascontiguousarray(w_v[r0:r0 + GD, :].T).astype(ndt),
                "woT": np.ascontiguousarray(w_o[:, r0:r0 + GD].T).astype(ndt),
                "tri": tri,
                "mskb": mskb,
            })
    return in_maps


def kernel(q, k, v, w_q, w_k, w_v, w_o):
    from concourse.bass_utils import run_bass_kernel_spmd

    dt_name = _DT
    nc = _get_program(dt_name)
    in_maps = make_in_maps(q, k, v, w_q, w_k, w_v, w_o, dt_name)
    res = run_bass_kernel_spmd(nc, in_maps, core_ids=list(range(N_CORES)))
    parts = [np.asarray(res.results[i]["outT"], dtype=np.float32)
             for i in range(N_CORES)]
    out = np.empty((B, N, D_MODEL), np.float32)
    for b in range(B):
        acc = parts[N_GROUPS * b]
        for g in range(1, N_GROUPS):
            acc = acc + parts[N_GROUPS * b + g]
        out[b] = acc.T
    return out
